# revision 2
# baseline (speedup 1.0000x reference)
"""Block-circulant linear layer (CirculantLinear) via frequency-domain block
matmul on 8 Trainium2 NeuronCores.

Math: the reference computes out = x @ W with W block-circulant,
W[x*8+m, y*8+j] = eigens[y, x, (j-m) % 8].  Diagonalizing every 8x8
circulant block by the length-8 DFT turns the dense [1024,1024] contraction
into 8 independent per-frequency [128(gx) -> 128(y)] matmuls -- complex for
k=1..3, real for k=0,4 -- i.e. 14 real 128x128 matmul planes instead of the
64 planes of the dense form: 4.6x less PE work and 8x less weight traffic.

Split of work:
  host   : length-8 real FFT of x (real-packed -- 8 real planes per 8
           inputs, exactly the same element count), weight FFT/packing,
           and the length-8 inverse FFT of the result.  All O(B*C*8)
           staging, same spirit as the dense baseline's host-side W
           expansion + transpose.
  device : the full [B,128]x[128,128]x14 frequency-domain contraction,
           data-parallel over batch on 8 cores; bf16 in / bf16 out to halve
           HBM traffic, fp32 PSUM accumulation (rel err ~2.9e-3).

Per-core HBM traffic: 8.39 MB in + 8.39 MB out + 0.36 MB weights = 17.2 MB,
which streams at the ~358 GB/s per-NC HBM limit in ~48 us; PE (~24 us warm)
and the eviction engines hide underneath.  Eviction detail: PSUM is drained
in two 4-bank groups per 512-row chunk with one wide cast each -- group A on
ScalarE (ACTIVATE Copy), group B on VectorE (TENSOR_SCALAR mul-by-1).
VectorE TENSOR_COPY with an fp32->bf16 PSUM read is avoided: it wedges the
exec unit on TRN2 hardware (NRT_EXEC_UNIT_UNRECOVERABLE).
"""

import sys

import numpy as np

_TRN = "/opt/trn_rl_repo"
if _TRN not in sys.path:
    sys.path.insert(0, _TRN)

# If the image's antenv lacks axon_hooks, stub it so bass_utils' trace
# path (taken when BASS_TRACE=1 is set in the environment) cannot crash.
try:
    import antenv.axon_hooks  # noqa: F401
except Exception:  # pragma: no cover
    import types

    _m = types.ModuleType("antenv.axon_hooks")
    _m._hook = None
    _m.set_axon_ntff_profile_hook = lambda h: setattr(_m, "_hook", h)
    _m.get_axon_ntff_profile_hook = lambda: getattr(_m, "_hook", None)
    sys.modules["antenv.axon_hooks"] = _m

import ml_dtypes

import concourse.bacc as bacc
import concourse.bass as bass
import concourse.mybir as mybir
from concourse.bass_utils import run_bass_kernel_spmd
from concourse.tile import TileContext

_dt = mybir.dt
_bf16 = ml_dtypes.bfloat16

N_CORES = 8
B, IN_CH, OUT_CH, MINI = 32768, 1024, 1024, 8
GY, GX = OUT_CH // MINI, IN_CH // MINI  # 128, 128
P = 128
BS = B // N_CORES    # rows per core (4096)
CH = 512             # batch columns per chunk (= one fp32 PSUM bank)
NC_CHUNK = BS // CH  # chunks per core (8)
NPL = 8              # real-packed frequency planes
NW = 11              # distinct weight planes: C0 C4 + (Ck Dk -Dk) x k=1..3

# weight plane order in the packed dram tensor
_W_C = {0: 0, 4: 1, 1: 2, 2: 5, 3: 8}
_W_D = {1: 3, 2: 6, 3: 9}
_W_ND = {1: 4, 2: 7, 3: 10}


def _fwd_pack_matrix() -> np.ndarray:
    """Tf [8(plane), 8(m)]: planes = x8 @ Tf.T (real-packed length-8 rfft)."""
    F = np.fft.rfft(np.eye(MINI), axis=-1)  # [m, 5] complex
    Tf = np.zeros((NPL, MINI))
    Tf[0] = F[:, 0].real
    for k in (1, 2, 3):
        Tf[2 * k - 1] = F[:, k].real
        Tf[2 * k] = F[:, k].imag
    Tf[7] = F[:, 4].real
    return Tf


def _inv_pack_matrix() -> np.ndarray:
    """Ti [8(j), 8(plane)]: out8 = q8 @ Ti.T (irfft of the real packing)."""
    Ti = np.zeros((MINI, NPL))
    for q in range(NPL):
        Y = np.zeros(5, np.complex128)
        if q == 0:
            Y[0] = 1.0
        elif q == 7:
            Y[4] = 1.0
        else:
            k = (q + 1) // 2
            Y[k] = 1.0 if q % 2 == 1 else 1.0j
        Ti[:, q] = np.fft.irfft(Y, n=MINI)
    return Ti


_TF = _fwd_pack_matrix()
_TI = _inv_pack_matrix()


def _pack_weights(eigens: np.ndarray) -> np.ndarray:
    """eigens [GY, GX, 8] -> packed lhsT planes [128, NW*128] bf16."""
    E = np.fft.fft(eigens.astype(np.float64), axis=-1)  # [y, gx, k]
    w = np.zeros((P, NW * P), np.float32)

    def put(i, m):
        w[:, i * P : (i + 1) * P] = m.astype(np.float32)

    put(_W_C[0], E[..., 0].real.T)
    put(_W_C[4], E[..., 4].real.T)
    for k in (1, 2, 3):
        put(_W_C[k], E[..., k].real.T)
        put(_W_D[k], E[..., k].imag.T)
        put(_W_ND[k], -E[..., k].imag.T)
    return w.astype(_bf16)


def _build_nc() -> bass.Bass:
    nc = bacc.Bacc()
    CW = NPL * CH  # columns per chunk (4096)
    xf_d = nc.declare_dram_parameter("xf", [P, NC_CHUNK * CW], _dt.bfloat16, isOutput=False)
    w_d = nc.declare_dram_parameter("w", [P, NW * P], _dt.bfloat16, isOutput=False)
    o_d = nc.declare_dram_parameter("out", [P, NC_CHUNK * CW], _dt.bfloat16, isOutput=True)

    with TileContext(nc) as tc:
        with (
            tc.tile_pool(name="wpool", bufs=1) as wpool,
            tc.tile_pool(name="xpool", bufs=4) as xpool,
            tc.tile_pool(name="opool", bufs=4) as opool,
            tc.tile_pool(name="pso", bufs=1, space="PSUM") as pso,
        ):
            # chunk 0 loads as two half-tiles so the first matmuls unblock
            # after 0.5 MB; weights go on the scalar HWDGE ring so they don't
            # head-of-line-block chunk 0 on the sync ring.
            sta = xpool.tile([P, 4 * CH], _dt.bfloat16, tag="sta", name="sta")
            nc.sync.dma_start(out=sta[:], in_=xf_d[:, 0 : 4 * CH])
            wt = wpool.tile([P, NW * P], _dt.bfloat16, name="wt")
            nc.scalar.dma_start(out=wt[:], in_=w_d[:, :])
            stb = xpool.tile([P, 4 * CH], _dt.bfloat16, tag="stb", name="stb")
            nc.sync.dma_start(out=stb[:], in_=xf_d[:, 4 * CH : CW])

            def wsl(i):
                return wt[:, i * P : (i + 1) * P]

            def load_chunk(c):
                t = xpool.tile([P, CW], _dt.bfloat16, tag="xin", name=f"xin{c}")
                nc.sync.dma_start(out=t[:], in_=xf_d[:, c * CW : (c + 1) * CW])
                return t

            tiles = {c: load_chunk(c) for c in (1, 2, 3, 4)}

            for c in range(NC_CHUNK):
                if c == 0:

                    def psl(p):
                        t = sta if p < 4 else stb
                        return t[:, (p % 4) * CH : (p % 4 + 1) * CH]

                else:
                    xin_t = tiles.pop(c)
                    if c + 4 < NC_CHUNK:
                        tiles[c + 4] = load_chunk(c + 4)

                    def psl(p, t=xin_t):
                        return t[:, p * CH : (p + 1) * CH]

                ot2 = opool.tile([P, CW], _dt.bfloat16, tag="ot", name=f"ot{c}")
                # out-plane q -> matmuls [(weight idx, in-plane p), ...]
                #   q: 0=Y0, 2k-1=ReYk, 2k=ImYk (k=1..3), 7=Y4
                mm_of = {
                    0: [(_W_C[0], 0)],
                    7: [(_W_C[4], 7)],
                }
                for k in (1, 2, 3):
                    mm_of[2 * k - 1] = [(_W_C[k], 2 * k - 1), (_W_ND[k], 2 * k)]
                    mm_of[2 * k] = [(_W_D[k], 2 * k - 1), (_W_C[k], 2 * k)]

                # two 4-bank PSUM groups; each drained by ONE wide cast op
                # (amortizes the per-instruction overhead 4x) -- group A on
                # ScalarE, group B on VectorE so they overlap.
                for gi in range(2):
                    po = pso.tile(
                        [P, 4 * CH], _dt.float32, tag=f"pg{gi}", name=f"pg{gi}_{c}"
                    )
                    for q in range(4 * gi, 4 * gi + 4):
                        sl = po[:, (q % 4) * CH : (q % 4 + 1) * CH]
                        mms = mm_of[q]
                        for j, (wi, p) in enumerate(mms):
                            nc.tensor.matmul(
                                sl,
                                lhsT=wsl(wi),
                                rhs=psl(p),
                                start=(j == 0),
                                stop=(j == len(mms) - 1),
                            )
                    dst = ot2[:, gi * 4 * CH : (gi + 1) * 4 * CH]
                    if gi == 1:
                        nc.vector.tensor_scalar_mul(dst, po[:], 1.0)
                    else:
                        nc.scalar.copy(dst, po[:])
                    if c == NC_CHUNK - 1:
                        # last chunk: store each half as soon as its cast
                        # lands so the final DMA tail is short
                        nc.scalar.dma_start(
                            out=o_d[:, c * CW + gi * 4 * CH : c * CW + (gi + 1) * 4 * CH],
                            in_=dst,
                        )
                if c < NC_CHUNK - 1:
                    nc.scalar.dma_start(
                        out=o_d[:, c * CW : (c + 1) * CW], in_=ot2[:]
                    )
    nc.compile()
    return nc


def _stage_inputs(x: np.ndarray) -> list[np.ndarray]:
    """x [B, 1024] fp32 -> per-core [128, chunk*plane*512] bf16 dram arrays."""
    planes = x.reshape(B, GX, MINI) @ _TF.T.astype(np.float32)  # [B, gx, plane]
    per_core = []
    for i in range(N_CORES):
        pc = planes[i * BS : (i + 1) * BS]          # [4096, 128, 8]
        pc = pc.reshape(NC_CHUNK, CH, GX, NPL)      # [c, b', gx, p]
        pc = pc.transpose(2, 0, 3, 1)               # [gx, c, p, b']
        per_core.append(np.ascontiguousarray(pc.reshape(P, -1)).astype(_bf16))
    return per_core


def _unstage_outputs(outs: list[np.ndarray]) -> np.ndarray:
    """per-core [128, chunk*plane*512] bf16 -> out [B, 1024] fp32."""
    qs = []
    for o in outs:
        oc = np.asarray(o).astype(np.float32).reshape(P, NC_CHUNK, NPL, CH)
        qs.append(oc.transpose(1, 3, 0, 2))         # [c, b', y, q]
    q = np.concatenate([a.reshape(BS, GY, NPL) for a in qs], axis=0)  # [B, y, q]
    out = q @ _TI.T.astype(np.float32)              # [B, y, j]
    return np.ascontiguousarray(out.reshape(B, OUT_CH), dtype=np.float32)


def _run(x: np.ndarray, eigens: np.ndarray, trace: bool = False):
    x = np.ascontiguousarray(x, dtype=np.float32)
    w = _pack_weights(np.asarray(eigens, dtype=np.float32))
    nc = _build_nc()
    xs = _stage_inputs(x)
    in_maps = [{"xf": xs[i], "w": w} for i in range(N_CORES)]
    res = run_bass_kernel_spmd(nc, in_maps, list(range(N_CORES)), trace=trace)
    out = _unstage_outputs([res.results[i]["out"] for i in range(N_CORES)])
    return out, res


def kernel(x: np.ndarray, eigens: np.ndarray) -> np.ndarray:
    out, _ = _run(x, eigens)
    return out


# revision 3
# speedup vs baseline: 1.0214x; 1.0214x over previous
"""Block-circulant linear layer (CirculantLinear) via frequency-domain block
matmul on 8 Trainium2 NeuronCores.

Math: the reference computes out = x @ W with W block-circulant,
W[x*8+m, y*8+j] = eigens[y, x, (j-m) % 8].  Diagonalizing every 8x8
circulant block by the length-8 DFT turns the dense [1024,1024] contraction
into 8 independent per-frequency [128(gx) -> 128(y)] matmuls -- complex for
k=1..3, real for k=0,4 -- i.e. 14 real 128x128 matmul planes instead of the
64 planes of the dense form: 4.6x less PE work and 8x less weight traffic.

Split of work:
  host   : length-8 real FFT of x (real-packed -- 8 real planes per 8
           inputs, exactly the same element count), weight FFT/packing,
           and the length-8 inverse FFT of the result.  All O(B*C*8)
           staging, same spirit as the dense baseline's host-side W
           expansion + transpose.
  device : the full [B,128]x[128,128]x14 frequency-domain contraction,
           data-parallel over batch on 8 cores; bf16 in / bf16 out to halve
           HBM traffic, fp32 PSUM accumulation (rel err ~2.9e-3).

Per-core HBM traffic: 8.39 MB in + 8.39 MB out + 0.36 MB weights = 17.2 MB,
which streams at the ~358 GB/s per-NC HBM limit in ~48 us; PE (~24 us warm)
and the eviction engines hide underneath.  Eviction detail: PSUM is drained
in two 4-bank groups per 512-row chunk with one wide cast each -- group A on
ScalarE (ACTIVATE Copy), group B on VectorE (TENSOR_SCALAR mul-by-1).
VectorE TENSOR_COPY with an fp32->bf16 PSUM read is avoided: it wedges the
exec unit on TRN2 hardware (NRT_EXEC_UNIT_UNRECOVERABLE).
"""

import sys

import numpy as np

_TRN = "/opt/trn_rl_repo"
if _TRN not in sys.path:
    sys.path.insert(0, _TRN)

# If the image's antenv lacks axon_hooks, stub it so bass_utils' trace
# path (taken when BASS_TRACE=1 is set in the environment) cannot crash.
try:
    import antenv.axon_hooks  # noqa: F401
except Exception:  # pragma: no cover
    import types

    _m = types.ModuleType("antenv.axon_hooks")
    _m._hook = None
    _m.set_axon_ntff_profile_hook = lambda h: setattr(_m, "_hook", h)
    _m.get_axon_ntff_profile_hook = lambda: getattr(_m, "_hook", None)
    sys.modules["antenv.axon_hooks"] = _m

import ml_dtypes

import concourse.bacc as bacc
import concourse.bass as bass
import concourse.mybir as mybir
from concourse.bass_utils import run_bass_kernel_spmd
from concourse.tile import TileContext

_dt = mybir.dt
_bf16 = ml_dtypes.bfloat16

N_CORES = 8
B, IN_CH, OUT_CH, MINI = 32768, 1024, 1024, 8
GY, GX = OUT_CH // MINI, IN_CH // MINI  # 128, 128
P = 128
BS = B // N_CORES    # rows per core (4096)
CH = 512             # batch columns per chunk (= one fp32 PSUM bank)
NC_CHUNK = BS // CH  # chunks per core (8)
NPL = 8              # real-packed frequency planes
NW = 11              # distinct weight planes: C0 C4 + (Ck Dk -Dk) x k=1..3

# weight plane order in the packed dram tensor
_W_C = {0: 0, 4: 1, 1: 2, 2: 5, 3: 8}
_W_D = {1: 3, 2: 6, 3: 9}
_W_ND = {1: 4, 2: 7, 3: 10}


def _fwd_pack_matrix() -> np.ndarray:
    """Tf [8(plane), 8(m)]: planes = x8 @ Tf.T (real-packed length-8 rfft)."""
    F = np.fft.rfft(np.eye(MINI), axis=-1)  # [m, 5] complex
    Tf = np.zeros((NPL, MINI))
    Tf[0] = F[:, 0].real
    for k in (1, 2, 3):
        Tf[2 * k - 1] = F[:, k].real
        Tf[2 * k] = F[:, k].imag
    Tf[7] = F[:, 4].real
    return Tf


def _inv_pack_matrix() -> np.ndarray:
    """Ti [8(j), 8(plane)]: out8 = q8 @ Ti.T (irfft of the real packing)."""
    Ti = np.zeros((MINI, NPL))
    for q in range(NPL):
        Y = np.zeros(5, np.complex128)
        if q == 0:
            Y[0] = 1.0
        elif q == 7:
            Y[4] = 1.0
        else:
            k = (q + 1) // 2
            Y[k] = 1.0 if q % 2 == 1 else 1.0j
        Ti[:, q] = np.fft.irfft(Y, n=MINI)
    return Ti


_TF = _fwd_pack_matrix()
_TI = _inv_pack_matrix()


def _pack_weights(eigens: np.ndarray) -> np.ndarray:
    """eigens [GY, GX, 8] -> packed lhsT planes [128, NW*128] bf16."""
    E = np.fft.fft(eigens.astype(np.float64), axis=-1)  # [y, gx, k]
    w = np.zeros((P, NW * P), np.float32)

    def put(i, m):
        w[:, i * P : (i + 1) * P] = m.astype(np.float32)

    put(_W_C[0], E[..., 0].real.T)
    put(_W_C[4], E[..., 4].real.T)
    for k in (1, 2, 3):
        put(_W_C[k], E[..., k].real.T)
        put(_W_D[k], E[..., k].imag.T)
        put(_W_ND[k], -E[..., k].imag.T)
    return w.astype(_bf16)


def _build_nc() -> bass.Bass:
    nc = bacc.Bacc()
    CW = NPL * CH  # columns per chunk (4096)
    xf_d = nc.declare_dram_parameter("xf", [P, NC_CHUNK * CW], _dt.bfloat16, isOutput=False)
    w_d = nc.declare_dram_parameter("w", [P, NW * P], _dt.bfloat16, isOutput=False)
    o_d = nc.declare_dram_parameter("out", [P, NC_CHUNK * CW], _dt.bfloat16, isOutput=True)

    with TileContext(nc) as tc:
        with (
            tc.tile_pool(name="wpool", bufs=1) as wpool,
            tc.tile_pool(name="xpool", bufs=5) as xpool,
            tc.tile_pool(name="opool", bufs=5) as opool,
            tc.tile_pool(name="pso", bufs=1, space="PSUM") as pso,
        ):
            # chunk 0 loads as two half-tiles so the first matmuls unblock
            # after 0.5 MB; weights go on the scalar HWDGE ring so they don't
            # head-of-line-block chunk 0 on the sync ring.
            sta = xpool.tile([P, 4 * CH], _dt.bfloat16, tag="sta", name="sta")
            nc.sync.dma_start(out=sta[:], in_=xf_d[:, 0 : 4 * CH])
            wt = wpool.tile([P, NW * P], _dt.bfloat16, name="wt")
            nc.scalar.dma_start(out=wt[:], in_=w_d[:, :])
            stb = xpool.tile([P, 4 * CH], _dt.bfloat16, tag="stb", name="stb")
            nc.sync.dma_start(out=stb[:], in_=xf_d[:, 4 * CH : CW])

            def wsl(i):
                return wt[:, i * P : (i + 1) * P]

            def load_chunk(c):
                t = xpool.tile([P, CW], _dt.bfloat16, tag="xin", name=f"xin{c}")
                nc.sync.dma_start(out=t[:], in_=xf_d[:, c * CW : (c + 1) * CW])
                return t

            tiles = {c: load_chunk(c) for c in (1, 2, 3, 4, 5)}

            for c in range(NC_CHUNK):
                if c == 0:

                    def psl(p):
                        t = sta if p < 4 else stb
                        return t[:, (p % 4) * CH : (p % 4 + 1) * CH]

                else:
                    xin_t = tiles.pop(c)
                    if c + 5 < NC_CHUNK:
                        tiles[c + 5] = load_chunk(c + 5)

                    def psl(p, t=xin_t):
                        return t[:, p * CH : (p + 1) * CH]

                ot2 = opool.tile([P, CW], _dt.bfloat16, tag="ot", name=f"ot{c}")
                # out-plane q -> matmuls [(weight idx, in-plane p), ...]
                #   q: 0=Y0, 2k-1=ReYk, 2k=ImYk (k=1..3), 7=Y4
                mm_of = {
                    0: [(_W_C[0], 0)],
                    7: [(_W_C[4], 7)],
                }
                for k in (1, 2, 3):
                    mm_of[2 * k - 1] = [(_W_C[k], 2 * k - 1), (_W_ND[k], 2 * k)]
                    mm_of[2 * k] = [(_W_D[k], 2 * k - 1), (_W_C[k], 2 * k)]

                # two 4-bank PSUM groups; each drained by ONE wide cast op
                # (amortizes the per-instruction overhead 4x) -- group A on
                # ScalarE, group B on VectorE so they overlap.
                for gi in range(2):
                    po = pso.tile(
                        [P, 4 * CH], _dt.float32, tag=f"pg{gi}", name=f"pg{gi}_{c}"
                    )
                    for q in range(4 * gi, 4 * gi + 4):
                        sl = po[:, (q % 4) * CH : (q % 4 + 1) * CH]
                        mms = mm_of[q]
                        for j, (wi, p) in enumerate(mms):
                            nc.tensor.matmul(
                                sl,
                                lhsT=wsl(wi),
                                rhs=psl(p),
                                start=(j == 0),
                                stop=(j == len(mms) - 1),
                            )
                    dst = ot2[:, gi * 4 * CH : (gi + 1) * 4 * CH]
                    if gi == 1:
                        nc.vector.tensor_scalar_mul(dst, po[:], 1.0)
                    else:
                        nc.scalar.copy(dst, po[:])
                    if c == NC_CHUNK - 1:
                        # last chunk: store each half as soon as its cast
                        # lands so the final DMA tail is short
                        nc.scalar.dma_start(
                            out=o_d[:, c * CW + gi * 4 * CH : c * CW + (gi + 1) * 4 * CH],
                            in_=dst,
                        )
                if c < NC_CHUNK - 1:
                    nc.scalar.dma_start(
                        out=o_d[:, c * CW : (c + 1) * CW], in_=ot2[:]
                    )
    nc.compile()
    return nc


def _stage_inputs(x: np.ndarray) -> list[np.ndarray]:
    """x [B, 1024] fp32 -> per-core [128, chunk*plane*512] bf16 dram arrays."""
    planes = x.reshape(B, GX, MINI) @ _TF.T.astype(np.float32)  # [B, gx, plane]
    per_core = []
    for i in range(N_CORES):
        pc = planes[i * BS : (i + 1) * BS]          # [4096, 128, 8]
        pc = pc.reshape(NC_CHUNK, CH, GX, NPL)      # [c, b', gx, p]
        pc = pc.transpose(2, 0, 3, 1)               # [gx, c, p, b']
        per_core.append(np.ascontiguousarray(pc.reshape(P, -1)).astype(_bf16))
    return per_core


def _unstage_outputs(outs: list[np.ndarray]) -> np.ndarray:
    """per-core [128, chunk*plane*512] bf16 -> out [B, 1024] fp32."""
    qs = []
    for o in outs:
        oc = np.asarray(o).astype(np.float32).reshape(P, NC_CHUNK, NPL, CH)
        qs.append(oc.transpose(1, 3, 0, 2))         # [c, b', y, q]
    q = np.concatenate([a.reshape(BS, GY, NPL) for a in qs], axis=0)  # [B, y, q]
    out = q @ _TI.T.astype(np.float32)              # [B, y, j]
    return np.ascontiguousarray(out.reshape(B, OUT_CH), dtype=np.float32)


def _run(x: np.ndarray, eigens: np.ndarray, trace: bool = False):
    x = np.ascontiguousarray(x, dtype=np.float32)
    w = _pack_weights(np.asarray(eigens, dtype=np.float32))
    nc = _build_nc()
    xs = _stage_inputs(x)
    in_maps = [{"xf": xs[i], "w": w} for i in range(N_CORES)]
    res = run_bass_kernel_spmd(nc, in_maps, list(range(N_CORES)), trace=trace)
    out = _unstage_outputs([res.results[i]["out"] for i in range(N_CORES)])
    return out, res


def kernel(x: np.ndarray, eigens: np.ndarray) -> np.ndarray:
    out, _ = _run(x, eigens)
    return out
